# revision 21
# baseline (speedup 1.0000x reference)
"""Multi-head attention (B=4, L=2048, D=1024, H=16) on 8 trn2 NeuronCores.

Sharding: core = b*2 + hg  ->  batch sample b in 0..3, head-group hg in 0..1.
Each core handles one batch sample and 8 heads (512 of the 1024 model dims):
data parallel on B, tensor parallel on H (column-split w_q/w_k/w_v, row-split
w_o).  Each core produces a partial output (its 8 heads' contribution through
w_o); the host sums the two head-group partials per sample.

Device-side layouts are all transposed (contraction dim on partitions) so the
kernel needs no on-device transposes:
  xT   [D=1024, L=2048]  (host pre-transposes q/k/v per sample)
  QT/KT [512, 2048]      head-pair tiles: partitions 0:64 head 2p, 64:128 head 2p+1
  V    natural [L, 512]  stored per l-tile as [128, 8 heads, 65] with a ones
                         column appended per head (row sums of P for free)
  S^T  [keys, queries]   per (pair, m-block) in PSUM -> exp on ACT -> P^T bf16
  O^T  = V^T @ P^T accumulated in PSUM; row 64 = sum_n P^T[n, m] (softmax denom)
  outT [1024, 2048]      final partial, host transposes back
Softmax skips max-subtraction: scores are ~N(0,1) (inputs randn, w scaled by
1/sqrt(D)); |score| < ~7 over 33M samples, far from fp32 exp overflow.
"""

import os
import numpy as np
import ml_dtypes
from contextlib import ExitStack

import concourse.bass as bass
import concourse.tile as tile
from concourse import bacc, mybir
from concourse.bass import ts
from concourse.bass_utils import run_bass_kernel_spmd

B, L, D, H = 4, 2048, 1024, 16
DK = 64          # head dim
HLOC = 8         # heads per core
DL = 512         # model dims per core (HLOC * DK)
SCALE = 1.0 / 8.0
NCORES = 8

COMPUTE_DT = os.environ.get("KDT", "bf16")   # 'bf16' | 'f32'
VARIANT = os.environ.get("KVARIANT", "full")  # 'full' | 'proj' | 'nopv'
KREP = int(os.environ.get("KREP", "1"))       # body repetitions (timing only)

_MDT = {"bf16": mybir.dt.bfloat16, "f32": mybir.dt.float32}
_NPDT = {"bf16": ml_dtypes.bfloat16, "f32": np.float32}


def _build(nc, l=L):
    cdt = _MDT[COMPUTE_DT]
    f32 = mybir.dt.float32
    nlt = l // 128       # l-tiles of 128
    nlb = l // 512       # l-blocks of 512
    ndt = D // 128       # contraction d-tiles for projections

    xqT = nc.dram_tensor("xqT", [D, l], cdt, kind="ExternalInput").ap()
    xkT = nc.dram_tensor("xkT", [D, l], cdt, kind="ExternalInput").ap()
    xvT = nc.dram_tensor("xvT", [D, l], cdt, kind="ExternalInput").ap()
    wqT = nc.dram_tensor("wqT", [D, DL], cdt, kind="ExternalInput").ap()
    wkT = nc.dram_tensor("wkT", [D, DL], cdt, kind="ExternalInput").ap()
    wvT = nc.dram_tensor("wvT", [D, DL], cdt, kind="ExternalInput").ap()
    woT = nc.dram_tensor("woT", [DL, D], cdt, kind="ExternalInput").ap()
    outT = nc.dram_tensor("outT", [D, l], f32, kind="ExternalOutput").ap()

    with tile.TileContext(nc) as tc, ExitStack() as ctx:
        persist = ctx.enter_context(tc.tile_pool(name="persist", bufs=1))
        wpool = ctx.enter_context(tc.tile_pool(name="wpool", bufs=2))
        xpool = ctx.enter_context(tc.tile_pool(name="xpool", bufs=2))
        ppool = ctx.enter_context(tc.tile_pool(name="ppool", bufs=20))
        otpool = ctx.enter_context(tc.tile_pool(name="otpool", bufs=12))
        opool = ctx.enter_context(tc.tile_pool(name="opool", bufs=3))
        small = ctx.enter_context(tc.tile_pool(name="small", bufs=4))
        ps_s = ctx.enter_context(tc.tile_pool(name="ps_s", bufs=2, space="PSUM"))
        ps_pv = ctx.enter_context(tc.tile_pool(name="ps_pv", bufs=2, space="PSUM"))
        ps_pr = ctx.enter_context(tc.tile_pool(name="ps_pr", bufs=2, space="PSUM"))

        # persistent tiles
        QT = [persist.tile([128, l], cdt, tag=f"qt{i}", name=f"qt{i}") for i in range(4)]
        KT = [persist.tile([128, l], cdt, tag=f"kt{i}", name=f"kt{i}") for i in range(4)]
        V = [persist.tile([128, HLOC, DK + 1], cdt, tag=f"v{i}", name=f"v{i}")
             for i in range(nlt)]
        WO = [persist.tile([64, D], cdt, tag=f"wo{i}", name=f"wo{i}") for i in range(HLOC)]

        # ones column per head in V (softmax denominator accumulates via PE)
        for lt in range(nlt):
            nc.gpsimd.memset(V[lt][:, :, DK:DK + 1], 1.0)
        for h in range(HLOC):
            nc.sync.dma_start(out=WO[h], in_=woT[h * DK:(h + 1) * DK, :])

        # ---- projections ----
        def project(which, xdram, wdram, lbs, rep=0):
            wt = []
            for t in range(ndt):
                w = wpool.tile([128, DL], cdt, tag=f"w{t}", name=f"w_{which}{t}_{rep}")
                nc.sync.dma_start(out=w, in_=wdram[t * 128:(t + 1) * 128, :])
                wt.append(w)
            for lb in lbs:
                xt = []
                for t in range(ndt):
                    x = xpool.tile([128, 512], cdt, tag=f"x{t}", name=f"x_{which}{lb}_{t}")
                    nc.sync.dma_start(out=x, in_=xdram[t * 128:(t + 1) * 128, ts(lb, 512)])
                    xt.append(x)
                if which in ("q", "k"):
                    dst = QT if which == "q" else KT
                    for et in range(4):
                        ps = ps_pr.tile([128, 512], f32, tag="pr", name=f"ps_{which}{lb}_{et}")
                        for t in range(ndt):
                            nc.tensor.matmul(ps, lhsT=wt[t][:, ts(et, 128)], rhs=xt[t],
                                             start=(t == 0), stop=(t == ndt - 1))
                        nc.vector.tensor_copy(dst[et][:, ts(lb, 512)], ps)
                else:
                    for j in range(4):
                        ps = ps_pr.tile([128, 512], f32, tag="pr", name=f"ps_v{lb}_{j}")
                        for t in range(ndt):
                            nc.tensor.matmul(ps, lhsT=xt[t][:, ts(j, 128)], rhs=wt[t],
                                             start=(t == 0), stop=(t == ndt - 1))
                        lt = lb * 4 + j
                        nc.vector.tensor_copy(
                            V[lt][:, :, 0:DK],
                            ps.rearrange("p (h d) -> p h d", h=HLOC))

        # ---- attention + output projection, per 512-query m-block ----
        ngrp = nlt // 2

        def s_phase(mb, p):
            """score matmuls + exp for one head-pair; needs only KT/QT."""
            for g in range(ngrp):
                sA = ps_s.tile([128, 1024], f32, tag="s", name=f"sA_{mb}_{p}_{g}")
                sB = ps_s.tile([128, 1024], f32, tag="s", name=f"sB_{mb}_{p}_{g}")
                for j in (0, 1):
                    nt = 2 * g + j
                    nc.tensor.matmul(sA[:, ts(j, 512)],
                                     lhsT=KT[p][0:64, ts(nt, 128)],
                                     rhs=QT[p][0:64, ts(mb, 512)],
                                     start=True, stop=True)
                    nc.tensor.matmul(sB[:, ts(j, 512)],
                                     lhsT=KT[p][64:128, ts(nt, 128)],
                                     rhs=QT[p][64:128, ts(mb, 512)],
                                     start=True, stop=True)
                pa = ppool.tile([128, 1024], cdt, tag="pt", name=f"pa_{mb}_{p}_{g}")
                pb = ppool.tile([128, 1024], cdt, tag="pt", name=f"pb_{mb}_{p}_{g}")
                nc.scalar.activation(pa, sA, mybir.ActivationFunctionType.Exp)
                nc.scalar.activation(pb, sB, mybir.ActivationFunctionType.Exp)
                yield g, pa, pb

        def pv_phase(mb, p, gps, ots):
            """P @ V accumulation + normalization; needs V tiles."""
            hA, hB = 2 * p, 2 * p + 1
            oA = ps_pv.tile([DK + 1, 512], f32, tag="pv", name=f"oA_{mb}_{p}")
            oB = ps_pv.tile([DK + 1, 512], f32, tag="pv", name=f"oB_{mb}_{p}")
            for g, pa, pb in gps:
                for j in (0, 1):
                    nt = 2 * g + j
                    nc.tensor.matmul(oA, lhsT=V[nt][:, hA, :], rhs=pa[:, ts(j, 512)],
                                     start=(nt == 0), stop=(nt == nlt - 1))
                    nc.tensor.matmul(oB, lhsT=V[nt][:, hB, :], rhs=pb[:, ts(j, 512)],
                                     start=(nt == 0), stop=(nt == nlt - 1))
            for o, h in ((oA, hA), (oB, hB)):
                r = small.tile([1, 512], f32, tag="r", name=f"r_{mb}_{h}")
                nc.vector.reciprocal(r, o[DK:DK + 1, :])
                rb = small.tile([64, 512], f32, tag="rb", name=f"rb_{mb}_{h}")
                nc.gpsimd.partition_broadcast(rb, r)
                ot = otpool.tile([64, 512], cdt, tag="ot", name=f"ot_{mb}_{h}")
                nc.vector.tensor_mul(ot, o[0:DK, :], rb)
                ots[h] = ot

        def attn_pairs(mb, plist, ots, rep=0):
            for p in plist:
                if VARIANT == "nopv":
                    for _ in s_phase(mb, p):
                        pass
                    continue
                pv_phase(mb, p, s_phase(mb, p), ots)

        def outproj(mb, ots):
            if VARIANT == "nopv":
                return
            for et in range(ndt):
                po = ps_pr.tile([128, 512], f32, tag="pr", name=f"po_{mb}_{et}")
                for h in range(HLOC):
                    nc.tensor.matmul(po, lhsT=WO[h][:, ts(et, 128)], rhs=ots[h],
                                     start=(h == 0), stop=(h == HLOC - 1))
                ost = opool.tile([128, 512], f32, tag="ostage", name=f"ost_{mb}_{et}")
                nc.vector.tensor_copy(ost, po)
                nc.sync.dma_start(out=outT[ts(et, 128), ts(mb, 512)], in_=ost)

        for rep in range(KREP):
            project("k", xkT, wkT, range(nlb), rep)
            project("q", xqT, wqT, [0], rep)
            if VARIANT == "proj":
                project("v", xvT, wvT, range(nlb), rep)
                for et in range(4):
                    ost = opool.tile([128, l], f32, tag="big", name=f"pst_{rep}_{et}")
                    nc.vector.tensor_copy(ost, QT[et])
                    nc.sync.dma_start(out=outT[ts(et, 128), :], in_=ost)
                continue
            # mb0-pair0 scores/exp are emitted before V-proj (they only need
            # K + Q-block0) so ACT starts early; their PV matmuls come after
            # V-proj, holding pair0's P tiles in the ppool meanwhile.
            ots = {mb: [None] * HLOC for mb in range(nlb)}
            gps0 = list(s_phase(0, 0))
            project("v", xvT, wvT, range(nlb), rep)
            if VARIANT != "nopv":
                pv_phase(0, 0, gps0, ots[0])
            attn_pairs(0, [1, 2, 3], ots[0], rep)
            for mb in range(nlb):
                if mb + 1 < nlb:
                    project("q", xqT, wqT, [mb + 1], rep)
                    # next block's first pair keeps ACT fed during out-proj
                    attn_pairs(mb + 1, [0], ots[mb + 1], rep)
                outproj(mb, ots[mb])
                if mb + 1 < nlb:
                    attn_pairs(mb + 1, [1, 2, 3], ots[mb + 1], rep)


_PROGRAM = None


def _get_program():
    global _PROGRAM
    if _PROGRAM is None:
        nc = bacc.Bacc("TRN2", target_bir_lowering=False, debug=False,
                       enable_asserts=False)
        _build(nc)
        nc.compile()
        _PROGRAM = nc
    return _PROGRAM


def _in_maps(q, k, v, w_q, w_k, w_v, w_o):
    npdt = _NPDT[COMPUTE_DT]
    q, k, v = (np.asarray(a, np.float32) for a in (q, k, v))
    w_q, w_k, w_v, w_o = (np.asarray(a, np.float32) for a in (w_q, w_k, w_v, w_o))
    maps = []
    for core in range(NCORES):
        b, hg = divmod(core, 2)
        hsl = slice(hg * DL, (hg + 1) * DL)
        maps.append({
            "xqT": np.ascontiguousarray(q[b].T).astype(npdt),
            "xkT": np.ascontiguousarray(k[b].T).astype(npdt),
            "xvT": np.ascontiguousarray(v[b].T).astype(npdt),
            "wqT": np.ascontiguousarray((w_q[hsl] * SCALE).T).astype(npdt),
            "wkT": np.ascontiguousarray(w_k[hsl].T).astype(npdt),
            "wvT": np.ascontiguousarray(w_v[hsl].T).astype(npdt),
            "woT": np.ascontiguousarray(w_o[:, hsl].T).astype(npdt),
        })
    return maps


def _run(inputs, **kwargs):
    nc = _get_program()
    maps = _in_maps(**inputs)
    res = run_bass_kernel_spmd(nc, maps, list(range(NCORES)), **kwargs)
    out = np.zeros((B, L, D), np.float32)
    for core in range(NCORES):
        out[core // 2] += res.results[core]["outT"].T
    return out, res


def kernel(q, k, v, w_q, w_k, w_v, w_o):
    out, _ = _run(dict(q=q, k=k, v=v, w_q=w_q, w_k=w_k, w_v=w_v, w_o=w_o))
    return out


# revision 31
# speedup vs baseline: 12.0553x; 12.0553x over previous
"""Multi-head attention (B=4, L=2048, D=1024, H=16) on 8 trn2 NeuronCores.

Sharding: core = b*2 + hg  ->  batch sample b in 0..3, head-group hg in 0..1.
Each core handles one batch sample and 8 heads (512 of the 1024 model dims):
data parallel on B, tensor parallel on H (column-split w_q/w_k/w_v, row-split
w_o).  Each core produces a partial output (its 8 heads' contribution through
w_o); the host sums the two head-group partials per sample.

Device-side layouts are all transposed (contraction dim on partitions) so the
kernel needs no on-device transposes:
  xT   [D=1024, L=2048]  (host pre-transposes q/k/v per sample)
  QT/KT [512, 2048]      head-pair tiles: partitions 0:64 head 2p, 64:128 head 2p+1
  V    natural [L, 512]  stored per l-tile as [128, 8 heads, 65] with a ones
                         column appended per head (row sums of P for free)
  S^T  [keys, queries]   per (pair, m-block) in PSUM -> exp on ACT -> P^T bf16
  O^T  = V^T @ P^T accumulated in PSUM; row 64 = sum_n P^T[n, m] (softmax denom)
  outT [1024, 2048]      final partial, host transposes back
Softmax skips max-subtraction: scores are ~N(0,1) (inputs randn, w scaled by
1/sqrt(D)); |score| < ~7 over 33M samples, far from fp32 exp overflow.
"""

import os
import numpy as np
import ml_dtypes
from contextlib import ExitStack

import concourse.bass as bass
import concourse.tile as tile
from concourse import bacc, mybir
from concourse.bass import ts
from concourse.bass_utils import run_bass_kernel_spmd

B, L, D, H = 4, 2048, 1024, 16
DK = 64          # head dim
HLOC = 8         # heads per core
DL = 512         # model dims per core (HLOC * DK)
SCALE = 1.0 / 8.0
NCORES = 8

COMPUTE_DT = os.environ.get("KDT", "bf16")   # 'bf16' | 'f32'
VARIANT = os.environ.get("KVARIANT", "full")  # 'full' | 'proj' | 'nopv'
KREP = int(os.environ.get("KREP", "1"))       # body repetitions (timing only)

_MDT = {"bf16": mybir.dt.bfloat16, "f32": mybir.dt.float32}
_NPDT = {"bf16": ml_dtypes.bfloat16, "f32": np.float32}


def _build(nc, l=L):
    cdt = _MDT[COMPUTE_DT]
    f32 = mybir.dt.float32
    nlt = l // 128       # l-tiles of 128
    nlb = l // 512       # l-blocks of 512
    ndt = D // 128       # contraction d-tiles for projections

    xqT = nc.dram_tensor("xqT", [D, l], cdt, kind="ExternalInput").ap()
    xkT = nc.dram_tensor("xkT", [D, l], cdt, kind="ExternalInput").ap()
    xvT = nc.dram_tensor("xvT", [D, l], cdt, kind="ExternalInput").ap()
    wqT = nc.dram_tensor("wqT", [D, DL], cdt, kind="ExternalInput").ap()
    wkT = nc.dram_tensor("wkT", [D, DL], cdt, kind="ExternalInput").ap()
    wvT = nc.dram_tensor("wvT", [D, DL], cdt, kind="ExternalInput").ap()
    woT = nc.dram_tensor("woT", [DL, D], cdt, kind="ExternalInput").ap()
    outT = nc.dram_tensor("outT", [D, l], f32, kind="ExternalOutput").ap()

    with tile.TileContext(nc) as tc, ExitStack() as ctx:
        persist = ctx.enter_context(tc.tile_pool(name="persist", bufs=1))
        wpool = ctx.enter_context(tc.tile_pool(name="wpool", bufs=2))
        xpool = ctx.enter_context(tc.tile_pool(name="xpool", bufs=2))
        ppool = ctx.enter_context(tc.tile_pool(name="ppool", bufs=20))
        otpool = ctx.enter_context(tc.tile_pool(name="otpool", bufs=7))
        opool = ctx.enter_context(tc.tile_pool(name="opool", bufs=3))
        small = ctx.enter_context(tc.tile_pool(name="small", bufs=4))
        ps_s = ctx.enter_context(tc.tile_pool(name="ps_s", bufs=2, space="PSUM"))
        ps_pv = ctx.enter_context(tc.tile_pool(name="ps_pv", bufs=2, space="PSUM"))
        ps_pr = ctx.enter_context(tc.tile_pool(name="ps_pr", bufs=2, space="PSUM"))

        # persistent tiles
        QT = [persist.tile([128, l], cdt, tag=f"qt{i}", name=f"qt{i}") for i in range(4)]
        KT = [persist.tile([128, l], cdt, tag=f"kt{i}", name=f"kt{i}") for i in range(4)]
        V = [persist.tile([128, HLOC, DK + 1], cdt, tag=f"v{i}", name=f"v{i}")
             for i in range(nlt)]
        # w_o as head-pair tiles [128, D] so the output projection contracts
        # with K=128 (full PE rows): pair p rows = heads 2p (0:64), 2p+1 (64:128)
        WO = [persist.tile([128, D], cdt, tag=f"wo{i}", name=f"wo{i}") for i in range(4)]

        # ones column per head in V (softmax denominator accumulates via PE)
        for lt in range(nlt):
            nc.gpsimd.memset(V[lt][:, :, DK:DK + 1], 1.0)
        for p in range(4):
            nc.sync.dma_start(out=WO[p], in_=woT[p * 128:(p + 1) * 128, :])

        # ---- projections ----
        def project(which, xdram, wdram, lbs, rep=0):
            wt = []
            for t in range(ndt):
                w = wpool.tile([128, DL], cdt, tag=f"w{t}", name=f"w_{which}{t}_{rep}")
                nc.sync.dma_start(out=w, in_=wdram[t * 128:(t + 1) * 128, :])
                wt.append(w)
            for lb in lbs:
                xt = []
                for t in range(ndt):
                    x = xpool.tile([128, 512], cdt, tag=f"x{t}", name=f"x_{which}{lb}_{t}")
                    nc.sync.dma_start(out=x, in_=xdram[t * 128:(t + 1) * 128, ts(lb, 512)])
                    xt.append(x)
                if which in ("q", "k"):
                    dst = QT if which == "q" else KT
                    for et in range(4):
                        ps = ps_pr.tile([128, 512], f32, tag="pr", name=f"ps_{which}{lb}_{et}")
                        for t in range(ndt):
                            nc.tensor.matmul(ps, lhsT=wt[t][:, ts(et, 128)], rhs=xt[t],
                                             start=(t == 0), stop=(t == ndt - 1))
                        nc.vector.tensor_copy(dst[et][:, ts(lb, 512)], ps)
                else:
                    for j in range(4):
                        ps = ps_pr.tile([128, 512], f32, tag="pr", name=f"ps_v{lb}_{j}")
                        for t in range(ndt):
                            nc.tensor.matmul(ps, lhsT=xt[t][:, ts(j, 128)], rhs=wt[t],
                                             start=(t == 0), stop=(t == ndt - 1))
                        lt = lb * 4 + j
                        nc.vector.tensor_copy(
                            V[lt][:, :, 0:DK],
                            ps.rearrange("p (h d) -> p h d", h=HLOC))

        # ---- attention + output projection, per 512-query m-block ----
        ngrp = nlt // 2

        def s_phase(mb, p):
            """score matmuls + exp for one head-pair; needs only KT/QT."""
            for g in range(ngrp):
                sA = ps_s.tile([128, 1024], f32, tag="s", name=f"sA_{mb}_{p}_{g}")
                sB = ps_s.tile([128, 1024], f32, tag="s", name=f"sB_{mb}_{p}_{g}")
                for j in (0, 1):
                    nt = 2 * g + j
                    # explicit row-group tile_position: the two K=64 head
                    # matmuls occupy complementary PE row halves and can run
                    # concurrently in the array
                    nc.tensor.matmul(sA[:, ts(j, 512)],
                                     lhsT=KT[p][0:64, ts(nt, 128)],
                                     rhs=QT[p][0:64, ts(mb, 512)],
                                     start=True, stop=True,
                                     tile_position=(0, 0))
                    nc.tensor.matmul(sB[:, ts(j, 512)],
                                     lhsT=KT[p][64:128, ts(nt, 128)],
                                     rhs=QT[p][64:128, ts(mb, 512)],
                                     start=True, stop=True,
                                     tile_position=(64, 0))
                pa = ppool.tile([128, 1024], cdt, tag="pt", name=f"pa_{mb}_{p}_{g}")
                pb = ppool.tile([128, 1024], cdt, tag="pt", name=f"pb_{mb}_{p}_{g}")
                nc.scalar.activation(pa, sA, mybir.ActivationFunctionType.Exp)
                nc.scalar.activation(pb, sB, mybir.ActivationFunctionType.Exp)
                yield g, pa, pb

        def pv_phase(mb, p, gps, ots):
            """P @ V accumulation + normalization; needs V tiles."""
            hA, hB = 2 * p, 2 * p + 1
            oA = ps_pv.tile([DK + 1, 512], f32, tag="pv", name=f"oA_{mb}_{p}")
            oB = ps_pv.tile([DK + 1, 512], f32, tag="pv", name=f"oB_{mb}_{p}")
            for g, pa, pb in gps:
                for j in (0, 1):
                    nt = 2 * g + j
                    nc.tensor.matmul(oA, lhsT=V[nt][:, hA, :], rhs=pa[:, ts(j, 512)],
                                     start=(nt == 0), stop=(nt == nlt - 1))
                    nc.tensor.matmul(oB, lhsT=V[nt][:, hB, :], rhs=pb[:, ts(j, 512)],
                                     start=(nt == 0), stop=(nt == nlt - 1))
            # assemble both heads into one [128, 512] O^T pair tile so the
            # output projection contracts at K=128: head A lands on
            # partitions 0:64 (DVE), head B is DMA-shifted to 64:128.
            otp = otpool.tile([128, 512], cdt, tag="ot", name=f"otp_{mb}_{p}")
            for o, h in ((oA, hA), (oB, hB)):
                r = small.tile([1, 512], f32, tag="r", name=f"r_{mb}_{h}")
                nc.vector.reciprocal(r, o[DK:DK + 1, :])
                rb = small.tile([64, 512], f32, tag="rb", name=f"rb_{mb}_{h}")
                nc.gpsimd.partition_broadcast(rb, r)
                if h == hA:
                    nc.vector.tensor_mul(otp[0:DK, :], o[0:DK, :], rb)
                else:
                    tmp = small.tile([64, 512], cdt, tag="obt", name=f"obt_{mb}_{h}")
                    nc.vector.tensor_mul(tmp, o[0:DK, :], rb)
                    nc.sync.dma_start(out=otp[64:128, :], in_=tmp)
            ots[p] = otp

        def attn_pairs(mb, plist, ots, rep=0):
            for p in plist:
                if VARIANT == "nopv":
                    for _ in s_phase(mb, p):
                        pass
                    continue
                pv_phase(mb, p, s_phase(mb, p), ots)

        def outproj(mb, ots):
            if VARIANT == "nopv":
                return
            for et in range(ndt):
                po = ps_pr.tile([128, 512], f32, tag="pr", name=f"po_{mb}_{et}")
                for p in range(4):
                    nc.tensor.matmul(po, lhsT=WO[p][:, ts(et, 128)], rhs=ots[p],
                                     start=(p == 0), stop=(p == 3))
                ost = opool.tile([128, 512], f32, tag="ostage", name=f"ost_{mb}_{et}")
                nc.vector.tensor_copy(ost, po)
                nc.sync.dma_start(out=outT[ts(et, 128), ts(mb, 512)], in_=ost)

        for rep in range(KREP):
            project("k", xkT, wkT, range(nlb), rep)
            project("q", xqT, wqT, [0], rep)
            if VARIANT == "proj":
                project("v", xvT, wvT, range(nlb), rep)
                for et in range(4):
                    ost = opool.tile([128, l], f32, tag="big", name=f"pst_{rep}_{et}")
                    nc.vector.tensor_copy(ost, QT[et])
                    nc.sync.dma_start(out=outT[ts(et, 128), :], in_=ost)
                continue
            # mb0-pair0 scores/exp are emitted before V-proj (they only need
            # K + Q-block0) so ACT starts early; their PV matmuls come after
            # V-proj, holding pair0's P tiles in the ppool meanwhile.
            ots = {mb: [None] * 4 for mb in range(nlb)}
            gps0 = list(s_phase(0, 0))
            project("v", xvT, wvT, range(nlb), rep)
            if VARIANT != "nopv":
                pv_phase(0, 0, gps0, ots[0])
            attn_pairs(0, [1, 2, 3], ots[0], rep)
            for mb in range(nlb):
                if mb + 1 < nlb:
                    project("q", xqT, wqT, [mb + 1], rep)
                    # next block's first pair keeps ACT fed during out-proj
                    attn_pairs(mb + 1, [0], ots[mb + 1], rep)
                outproj(mb, ots[mb])
                if mb + 1 < nlb:
                    attn_pairs(mb + 1, [1, 2, 3], ots[mb + 1], rep)


_PROGRAM = None


def _get_program():
    global _PROGRAM
    if _PROGRAM is None:
        nc = bacc.Bacc("TRN2", target_bir_lowering=False, debug=False,
                       enable_asserts=False)
        _build(nc)
        nc.compile()
        _PROGRAM = nc
    return _PROGRAM


def _in_maps(q, k, v, w_q, w_k, w_v, w_o):
    npdt = _NPDT[COMPUTE_DT]
    q, k, v = (np.asarray(a, np.float32) for a in (q, k, v))
    w_q, w_k, w_v, w_o = (np.asarray(a, np.float32) for a in (w_q, w_k, w_v, w_o))
    maps = []
    for core in range(NCORES):
        b, hg = divmod(core, 2)
        hsl = slice(hg * DL, (hg + 1) * DL)
        maps.append({
            "xqT": np.ascontiguousarray(q[b].T).astype(npdt),
            "xkT": np.ascontiguousarray(k[b].T).astype(npdt),
            "xvT": np.ascontiguousarray(v[b].T).astype(npdt),
            "wqT": np.ascontiguousarray((w_q[hsl] * SCALE).T).astype(npdt),
            "wkT": np.ascontiguousarray(w_k[hsl].T).astype(npdt),
            "wvT": np.ascontiguousarray(w_v[hsl].T).astype(npdt),
            "woT": np.ascontiguousarray(w_o[:, hsl].T).astype(npdt),
        })
    return maps


def _run(inputs, **kwargs):
    nc = _get_program()
    maps = _in_maps(**inputs)
    res = run_bass_kernel_spmd(nc, maps, list(range(NCORES)), **kwargs)
    out = np.zeros((B, L, D), np.float32)
    for core in range(NCORES):
        out[core // 2] += res.results[core]["outT"].T
    return out, res


def kernel(q, k, v, w_q, w_k, w_v, w_o):
    out, _ = _run(dict(q=q, k=k, v=v, w_q=w_q, w_k=w_k, w_v=w_v, w_o=w_o))
    return out
